# revision 18
# baseline (speedup 1.0000x reference)
"""GATConv (PyG defaults, heads=1) Trainium2 Bass kernel.

Strategy (8 NeuronCores, node-parallel over destinations, no collectives):
  - Host: prepend self-loops, sort edges by destination, partition the
    destination space into blocks of 128 nodes. Within a block, slot each
    edge at (chunk k, partition dst%128): the per-chunk attention weight
    matrix is DIAGONAL, so per-edge destination indexing is free
    (partition index == dst-local index). Self-loops sort first in each
    run, so chunk 0 holds h[dst] rows, from which a_d is recomputed.
  - Each core owns ceil(NB/8) dst blocks and all edges into them; output
    row ranges are disjoint, so results are just concatenated.
  - Device, per core:
      Phase 1: h = x @ W (from host-transposed x^T), a_s = h @ att_src;
               write augmented node table rows [h(128) | a_s | pad]
               (192 f32 = 768 B) to DRAM.
      Phase 2: per block: dma_gather table rows for all edge slots;
               a_d = (chunk-0 rows) @ att_dst; z = a_s[src] + a_d;
               ex = exp(leaky_relu(z)); lhsT = diag(ex) via iota-compare;
               PSUM += diag(ex) @ h_rows over chunks; denom = row-sum of
               ex; out = psum * (1/denom) + bias.
  - Softmax is unshifted (no segment max): |logits| <~ 12 for this data
    distribution so exp() is far from fp32 overflow, and alpha is
    shift-invariant, matching the reference to fp32 noise.
  - Padded slots gather a sentinel row with a_s = -1e30 -> ex = 0 exactly,
    contributing nothing to numerator or denominator.
"""

import os
import sys

import numpy as np

sys.path.insert(0, "/opt/trn_rl_repo")

P = 128
ROWB = 192          # table row width in f32 (768 B; dma_gather needs %256B==0)
A_S_COL = P         # column holding a_s inside a table row
NEG_SLOPE = 0.2
NCORES = 8


def build_program(NPAD, NB, BPC, K, SEG, L16, dummy_part):
    import os as _os
    _STAGE = _os.environ.get("GAT_STAGE", "full")
    from concourse import bacc, bass, mybir, tile

    f32 = mybir.dt.float32
    i16 = mybir.dt.int16
    Alu = mybir.AluOpType
    Act = mybir.ActivationFunctionType
    NSEG = K // SEG

    nc = bacc.Bacc(None, num_swdge_queues=4)

    xT = nc.declare_dram_parameter("xT", [P, NPAD], f32, isOutput=False)
    Wp = nc.declare_dram_parameter("W", [P, P], f32, isOutput=False)
    asr = nc.declare_dram_parameter("att_src_rep", [P, P], f32, isOutput=False)
    adr = nc.declare_dram_parameter("att_dst_rep", [P, P], f32, isOutput=False)
    brp = nc.declare_dram_parameter("bias_rep", [P, P], f32, isOutput=False)
    idxp = nc.declare_dram_parameter("idxs", [P, L16], i16, isOutput=False)
    outp = nc.declare_dram_parameter("out", [BPC * P, P], f32, isOutput=True)
    table = nc.dram_tensor("table", [NPAD, ROWB], f32)

    with tile.TileContext(nc) as tc:
        with (
            tc.tile_pool(name="const", bufs=1) as cpool,
            tc.tile_pool(name="ps1", bufs=4, space="PSUM") as ps1,
            tc.tile_pool(name="junk", bufs=2) as jpool,
            tc.tile_pool(name="gseg", bufs=3) as gpool,
            tc.tile_pool(name="exz", bufs=2) as epool,
            tc.tile_pool(name="diag", bufs=4) as dpool,
            tc.tile_pool(name="ps2", bufs=2, space="PSUM") as ps2,
            tc.tile_pool(name="outb", bufs=2) as opool,
        ):
            # ---- constants / inputs resident in SBUF ----
            xT_sb = cpool.tile([P, NPAD], f32)
            nc.sync.dma_start(out=xT_sb[:], in_=xT[:])
            W_sb = cpool.tile([P, P], f32)
            nc.sync.dma_start(out=W_sb[:], in_=Wp[:])
            asr_sb = cpool.tile([P, P], f32)
            nc.sync.dma_start(out=asr_sb[:], in_=asr[:])
            adr_sb = cpool.tile([P, P], f32)
            nc.sync.dma_start(out=adr_sb[:], in_=adr[:])
            brp_sb = cpool.tile([P, P], f32)
            nc.sync.dma_start(out=brp_sb[:], in_=brp[:])
            idx_sb = cpool.tile([P, L16], i16)
            nc.sync.dma_start(out=idx_sb[:], in_=idxp[:])

            iota_row = cpool.tile([P, P], f32)
            nc.gpsimd.iota(iota_row[:], pattern=[[1, P]], base=0,
                           channel_multiplier=0,
                           allow_small_or_imprecise_dtypes=True)
            iota_col = cpool.tile([P, 1], f32)
            nc.gpsimd.iota(iota_col[:], pattern=[[1, 1]], base=0,
                           channel_multiplier=1,
                           allow_small_or_imprecise_dtypes=True)

            # ---- phase 1: h = x @ W, a_s; write node table (full rows) ----
            for nb in range(NB):
                ph = ps1.tile([P, P], f32, tag="ph")
                nc.tensor.matmul(out=ph[:], lhsT=xT_sb[:, nb * P:(nb + 1) * P],
                                 rhs=W_sb[:], start=True, stop=True)
                hsb = jpool.tile([P, ROWB], f32, tag="hsb")
                t0 = jpool.tile([P, P], f32, tag="t0")
                nc.vector.scalar_tensor_tensor(
                    out=t0[:], in0=ph[:], scalar=1.0, in1=asr_sb[:],
                    op0=Alu.mult, op1=Alu.mult,
                    accum_out=hsb[:, A_S_COL:A_S_COL + 1])
                nc.scalar.activation(out=hsb[:, 0:P], in_=ph[:], func=Act.Copy)
                nc.gpsimd.memset(hsb[:, A_S_COL + 1:ROWB], 0.0)
                if nb == NB - 1:
                    # dummy node: h-row is zero (xT zero-padded), so its
                    # accumulated a_s is 0; add -1e30 at its partition so
                    # padded slots' exp() underflows to exactly 0.
                    fix = jpool.tile([P, 1], f32, tag="fix")
                    nc.vector.tensor_scalar(
                        fix[:], iota_col[:, 0:1], float(dummy_part), -1e30,
                        Alu.is_equal, Alu.mult)
                    nc.vector.tensor_tensor(
                        out=hsb[:, A_S_COL:A_S_COL + 1],
                        in0=hsb[:, A_S_COL:A_S_COL + 1], in1=fix[:],
                        op=Alu.add)
                nc.sync.dma_start(out=table[nb * P:(nb + 1) * P, :],
                                  in_=hsb[:])

            # ---- phase 2: per-block gather + attention + aggregation ----
            for j in range(BPC if _STAGE != "phase1" else 0):
                po = ps2.tile([P, P], f32, tag="po")
                ex_blk = epool.tile([P, K, 1], f32, tag="ex")
                ad_col = epool.tile([P, 1], f32, tag="adc")
                for s in range(NSEG):
                    g = gpool.tile([P, SEG, ROWB], f32, tag="g")
                    c16 = (j * K + s * SEG) * P // 16
                    nc.gpsimd.dma_gather(
                        out_ap=g[:], in_ap=table[:],
                        idxs_ap=idx_sb[:, c16:c16 + SEG * P // 16],
                        num_idxs=SEG * P, num_idxs_reg=SEG * P,
                        elem_size=ROWB, single_packet=False,
                        queue_num=(j * NSEG + s) % 4)
                    if s == 0:
                        # chunk 0 is the self-loop chunk: rows are h[dst]
                        if _STAGE == "noad":
                            nc.vector.tensor_scalar(
                                ad_col[:], iota_col[:, 0:1], 0.0, None,
                                Alu.mult)
                        else:
                            tj = jpool.tile([P, P], f32, tag="t0")
                            nc.vector.scalar_tensor_tensor(
                                out=tj[:], in0=g[:, 0, 0:P], scalar=1.0,
                                in1=adr_sb[:], op0=Alu.mult, op1=Alu.mult,
                                accum_out=ad_col[:])
                    z = epool.tile([P, SEG, 1], f32, tag="z")
                    nc.vector.tensor_scalar(
                        z[:], g[:, :, A_S_COL:A_S_COL + 1],
                        ad_col[:, 0:1], None, Alu.add)
                    lz = epool.tile([P, SEG, 1], f32, tag="lz")
                    nc.vector.scalar_tensor_tensor(
                        out=lz[:], in0=z[:], scalar=NEG_SLOPE, in1=z[:],
                        op0=Alu.mult, op1=Alu.max)
                    nc.scalar.activation(
                        out=ex_blk[:, s * SEG:(s + 1) * SEG, :],
                        in_=lz[:], func=Act.Exp)
                    for k in range(SEG):
                        c = s * SEG + k
                        dg = dpool.tile([P, P], f32, tag="dg")
                        nc.vector.tensor_scalar(
                            dg[:], iota_row[:], iota_col[:, 0:1],
                            ex_blk[:, c:c + 1, 0:1], Alu.is_equal, Alu.mult)
                        nc.tensor.matmul(out=po[:], lhsT=dg[:],
                                         rhs=g[:, k, 0:P],
                                         start=(c == 0), stop=(c == K - 1))
                # normalize + bias
                dn = epool.tile([P, 1], f32, tag="dn")
                nc.vector.tensor_reduce(out=dn[:], in_=ex_blk[:],
                                        axis=mybir.AxisListType.XY,
                                        op=Alu.add)
                dn2 = epool.tile([P, 1], f32, tag="dn2")
                nc.vector.tensor_scalar(dn2[:], dn[:], 1e-30, None, Alu.max)
                rc = epool.tile([P, 1], f32, tag="rc")
                nc.vector.reciprocal(out=rc[:], in_=dn2[:])
                ob = opool.tile([P, P], f32, tag="ob")
                nc.vector.scalar_tensor_tensor(
                    out=ob[:], in0=po[:], scalar=rc[:, 0:1], in1=brp_sb[:],
                    op0=Alu.mult, op1=Alu.add)
                nc.sync.dma_start(out=outp[j * P:(j + 1) * P, :], in_=ob[:])

            if _STAGE == "phase1":
                zb = opool.tile([P, P], f32, tag="ob")
                nc.vector.tensor_scalar(zb[:], brp_sb[:], 1.0, None, Alu.mult)
                for j in range(BPC):
                    nc.sync.dma_start(out=outp[j * P:(j + 1) * P, :], in_=zb[:])

    nc.compile()
    return nc


def prepare(x, W, att_src, att_dst, bias, edge_index):
    """Host-side sharding/slotting. Returns (program args, per-core in_maps)."""
    x = np.asarray(x, dtype=np.float32)
    W = np.asarray(W, dtype=np.float32)
    att_src = np.asarray(att_src, dtype=np.float32)
    att_dst = np.asarray(att_dst, dtype=np.float32)
    bias = np.asarray(bias, dtype=np.float32)
    ei = np.asarray(edge_index)

    N, D = x.shape
    assert D == P

    # self-loops FIRST so they land at chunk 0 of every destination run
    loop = np.arange(N, dtype=np.int64)
    src = np.concatenate([loop, ei[0]]).astype(np.int32)
    dst = np.concatenate([loop, ei[1]]).astype(np.int32)
    order = np.argsort(dst, kind="stable")
    src_s, dst_s = src[order], dst[order]

    NB = (N + P - 1) // P
    if NB * P == N:        # need a spare row for the dummy/sentinel node
        NB += 1
    NPAD = NB * P
    BPC = (NB + NCORES - 1) // NCORES

    deg = np.bincount(dst_s, minlength=NPAD)
    Kraw = max(int(deg.max()), 1)
    NSEG = max(1, (Kraw + 25) // 26)   # cap SEG at 26 chunks per gather
    SEG = (Kraw + NSEG - 1) // NSEG
    K = NSEG * SEG

    DUMMY = N
    assert DUMMY < NPAD
    dummy_part = DUMMY - (NB - 1) * P

    grid = np.full((NB, K, P), DUMMY, dtype=np.int16)
    runstart = np.zeros(NPAD, dtype=np.int64)
    runstart[1:] = np.cumsum(deg)[:-1]
    k_e = np.arange(len(dst_s), dtype=np.int64) - runstart[dst_s]
    grid[dst_s // P, k_e, dst_s % P] = src_s

    L = BPC * K * P
    L16 = L // 16
    idx_inputs = []
    for c in range(NCORES):
        flat = np.full((BPC, K, P), DUMMY, dtype=np.int16)
        b0 = c * BPC
        nreal = max(0, min(BPC, NB - b0))
        if nreal > 0:
            flat[:nreal] = grid[b0:b0 + nreal]
        wrapped = flat.reshape(-1, 16).T.copy()
        # the 8 GPSIMD Q7 cores each read indices from their own group of
        # 16 partitions -> replicate the wrapped block into every group
        full = np.empty((P, L16), dtype=np.int16)
        for gp in range(P // 16):
            full[16 * gp:16 * (gp + 1)] = wrapped
        idx_inputs.append(full)

    xT = np.zeros((P, NPAD), dtype=np.float32)
    xT[:, :N] = x.T
    asr = np.broadcast_to(att_src, (P, P)).copy()
    adr = np.broadcast_to(att_dst, (P, P)).copy()
    brp = np.broadcast_to(bias, (P, P)).copy()

    in_maps = [{"xT": xT, "W": W, "att_src_rep": asr, "att_dst_rep": adr,
                "bias_rep": brp, "idxs": idx_inputs[c]} for c in range(NCORES)]
    return (NPAD, NB, BPC, K, SEG, L16, dummy_part), in_maps, (N, D)


def kernel(x, W, att_src, att_dst, bias, edge_index):
    from concourse.bass_utils import run_bass_kernel_spmd

    args, in_maps, (N, D) = prepare(x, W, att_src, att_dst, bias, edge_index)
    nc = build_program(*args)
    res = run_bass_kernel_spmd(nc, in_maps, list(range(NCORES)))

    BPC = args[2]
    out = np.empty((N, D), dtype=np.float32)
    for c in range(NCORES):
        rows0 = c * BPC * P
        rows1 = min(rows0 + BPC * P, N)
        if rows1 > rows0:
            out[rows0:rows1] = res.results[c]["out"][:rows1 - rows0]
    return out
